# revision 1
# baseline (speedup 1.0000x reference)
"""Trainium2 kernel for nn_AttentionSparseMask.

Strategy: 8 NeuronCores, data-parallel over (batch n in {0,1}) x (hash round h in
{0..3}).  The dominant compute — the LSH-chunked sparse attention (S = Q@K^T,
exp, P@V and row sums; ~30 GFLOP) — runs on the NeuronCores via a Bass/Tile
kernel.  The host prepares the sorted/gathered operands (cheap, bandwidth-only)
and applies the small surrounding convolutions.
"""

import numpy as np
import ml_dtypes

BF16 = ml_dtypes.bfloat16

C = 64
RED = 4
CR = C // RED          # 16
N_HASHES = 4
CHUNK = 512
RES_SCALE = 0.1
EPS = 5e-5
H = W = 128
L = H * W              # 16384
NCH = L // CHUNK       # 32 chunks
KW = L + 2 * CHUNK     # wrapped key length 17408
NCORES = 8

_compiled = None


# ----------------------------------------------------------------- host convs
def conv1x1(x, w, b=None):
    # x [B,Ci,H,W], w [Co,Ci,1,1]
    out = np.einsum('oc,bchw->bohw', w[:, :, 0, 0], x, dtype=np.float32)
    if b is not None:
        out = out + b[None, :, None, None]
    return out.astype(np.float32)


def dwconv(x, w, b, pad):
    # depthwise conv, groups == channels. x [B,Cc,H,W], w [Cc,1,k,k]
    Bb, Cc, Hh, Ww = x.shape
    k = w.shape[2]
    xp = np.pad(x, ((0, 0), (0, 0), (pad, pad), (pad, pad)))
    out = np.zeros((Bb, Cc, Hh + 2 * pad - k + 1, Ww + 2 * pad - k + 1), np.float32)
    for dy in range(k):
        for dx in range(k):
            out += w[None, :, 0, dy, dx, None, None] * \
                xp[:, :, dy:dy + out.shape[2], dx:dx + out.shape[3]]
    if b is not None:
        out = out + b[None, :, None, None]
    return out


def ds_conv(x, pw_w, dw_w, dw_b, pad):
    return dwconv(conv1x1(x, pw_w), dw_w, dw_b, pad)


def pool2(x, mode):
    Bb, Cc, Hh, Ww = x.shape
    xr = x.reshape(Bb, Cc, Hh // 2, 2, Ww // 2, 2)
    return xr.max(axis=(3, 5)) if mode == 'max' else xr.mean(axis=(3, 5), dtype=np.float32)


def bilinear_ac(x, out_h, out_w):
    Bb, Cc, h, w = x.shape
    def coords(n_in, n_out):
        pos = (np.arange(n_out, dtype=np.float32) * np.float32((n_in - 1) / (n_out - 1)))
        lo = np.floor(pos).astype(np.int32)
        hi = np.minimum(lo + 1, n_in - 1)
        frac = (pos - lo.astype(np.float32)).astype(np.float32)
        return lo, hi, frac
    lo_h, hi_h, fh = coords(h, out_h)
    x = x[:, :, lo_h, :] * (1 - fh)[None, None, :, None] + x[:, :, hi_h, :] * fh[None, None, :, None]
    lo_w, hi_w, fw = coords(w, out_w)
    x = x[:, :, :, lo_w] * (1 - fw) + x[:, :, :, hi_w] * fw
    return x.astype(np.float32)


def sigmoid(x):
    return (1.0 / (1.0 + np.exp(-x.astype(np.float32)))).astype(np.float32)


# ------------------------------------------------------------- device kernel
def build_bass():
    import concourse.bass as bass
    import concourse.mybir as mybir
    import concourse.tile as tile
    from concourse import bacc

    nc = bacc.Bacc("TRN2", target_bir_lowering=False)
    f32 = mybir.dt.float32
    bf16 = mybir.dt.bfloat16

    qt_d = nc.dram_tensor("qt", [CR, L], bf16, kind="ExternalInput")
    kt_d = nc.dram_tensor("kt", [CR, KW], bf16, kind="ExternalInput")
    v3_d = nc.dram_tensor("v3", [KW, C + 1], bf16, kind="ExternalInput")
    evt_d = nc.dram_tensor("evt", [C + 1, L], f32, kind="ExternalOutput")

    NT = KW // 128  # 136 v-tiles

    with tile.TileContext(nc) as tc:
        with (
            tc.tile_pool(name="const", bufs=1) as cpool,
            tc.tile_pool(name="ps", bufs=3, space="PSUM") as pspool,
            tc.tile_pool(name="pr", bufs=2, space="PSUM") as prpool,
            tc.tile_pool(name="pt", bufs=18) as ptpool,
            tc.tile_pool(name="ev", bufs=3) as evpool,
        ):
            # kt/qt replicated into 4 partition strips (rows 32r..32r+16) so
            # the K=16 matmuls pack 4-up via tile_position row groups.
            qt = cpool.tile([128, L], bf16, tag="qt")
            kt = cpool.tile([128, KW], bf16, tag="kt")
            # v3 split into two tiles so the early chunks' matmul2 only
            # depends on the small first piece, not the whole 2.3MB load.
            NTA = 40   # covers chunks 0..7 (tiles c*4 .. c*4+11)
            v3a = cpool.tile([128, NTA, C + 1], bf16, tag="v3a")
            v3b = cpool.tile([128, NT - NTA, C + 1], bf16, tag="v3b")
            v3r = v3_d.rearrange("(t p) c -> p t c", p=128)
            # Split the replicated loads across both DGE paths (HWDGE via
            # sync + SWDGE via gpsimd) so the load phase halves.
            for r in range(4):
                eng = nc.sync if r % 2 == 0 else nc.gpsimd
                eng.dma_start(out=qt[32 * r:32 * r + CR, :], in_=qt_d[:])
                eng = nc.gpsimd if r % 2 == 0 else nc.sync
                eng.dma_start(out=kt[32 * r:32 * r + CR, :], in_=kt_d[:])
            nc.sync.dma_start(out=v3a[:], in_=v3r[:, :NTA, :])
            nc.sync.dma_start(out=v3b[:], in_=v3r[:, NTA:, :])

            def v3_tile(idx):
                return v3a[:, idx, :] if idx < NTA else v3b[:, idx - NTA, :]

            def emit_mm2(c, pts):
                pr = prpool.tile([C + 1, 512], f32, tag="pr")
                for kb in range(12):
                    rhs_ap = pts[kb // 2][:, (kb % 2) * 512:(kb % 2 + 1) * 512]
                    if rhs_ap.dtype == mybir.dt.int16:
                        rhs_ap = rhs_ap.bitcast(bf16)
                    nc.tensor.matmul(
                        out=pr[:],
                        lhsT=v3_tile(c * 4 + kb),
                        rhs=rhs_ap,
                        start=(kb == 0), stop=(kb == 11),
                    )
                ev = evpool.tile([C + 1, 512], f32, tag="ev")
                nc.vector.tensor_copy(ev[:], pr[:])
                nc.gpsimd.dma_start(out=evt_d[:, c * 512:(c + 1) * 512], in_=ev[:])

            prev_pts = None
            for c in range(NCH):
                pts = []
                for g in range(6):
                    ps = pspool.tile([128, 1024], f32, tag="ps")
                    for j in range(2):
                        kb = g * 2 + j
                        r = kb % 4
                        nc.tensor.matmul(
                            out=ps[:, j * 512:(j + 1) * 512],
                            lhsT=kt[32 * r:32 * r + CR,
                                    c * 512 + kb * 128: c * 512 + (kb + 1) * 128],
                            rhs=qt[32 * r:32 * r + CR, c * 512:(c + 1) * 512],
                            start=True, stop=True,
                            tile_position=(32 * r, 0),
                        )
                    dve_groups = (1, 4) if c % 2 == 0 else (0, 2, 4)
                    if g in dve_groups:
                        # DVE-assisted exp (Schraudolph bit-trick): y =
                        # round(x*128/ln2 + (127*128 - 7.4)) as int16 IS the
                        # bf16 bit pattern of ~exp(x) (raw is bounded in
                        # [-8, 10], so no overflow/sign issues).  Offloads
                        # 1/3 of the exp work from the ACT engine (the
                        # bottleneck) to the otherwise-idle DVE.
                        pti = ptpool.tile([128, 1024], mybir.dt.int16, tag="pt")
                        nc.vector.tensor_scalar(
                            out=pti[:], in0=ps[:],
                            scalar1=184.6649652, scalar2=16249.1,
                            op0=mybir.AluOpType.mult, op1=mybir.AluOpType.add,
                        )
                        pts.append(pti)
                    else:
                        pt = ptpool.tile([128, 1024], bf16, tag="pt")
                        nc.scalar.activation(pt[:], ps[:], mybir.ActivationFunctionType.Exp)
                        pts.append(pt)
                if prev_pts is not None:
                    emit_mm2(c - 1, prev_pts)
                prev_pts = pts
            emit_mm2(NCH - 1, prev_pts)
    nc.finalize()
    return nc


def get_compiled():
    global _compiled
    if _compiled is None:
        _compiled = build_bass()
    return _compiled


# ------------------------------------------------------------------- kernel
def kernel(trace=False, **inputs):
    x = np.asarray(inputs['x'], np.float32)
    B = x.shape[0]

    # --- MultiScaleSpatialAttention (host, ~50 MFLOP) ---
    xr = conv1x1(x, inputs['spa_down_w'], inputs['spa_down_b'])
    s0 = conv1x1(xr, inputs['s0_pw_w'])
    s0 = s0 * inputs['s0_dw_w'][None, :, 0, 0, 0, None, None] + inputs['s0_dw_b'][None, :, None, None]
    feats = [s0]
    for pw, dw, db, pad in ((inputs['br3_pw_w'], inputs['br3_dw_w'], inputs['br3_dw_b'], 1),
                            (inputs['br5_pw_w'], inputs['br5_dw_w'], inputs['br5_dw_b'], 2),
                            (inputs['br7_pw_w'], inputs['br7_dw_w'], inputs['br7_dw_b'], 3)):
        mx = ds_conv(pool2(xr, 'max'), pw, dw, db, pad)
        av = ds_conv(pool2(xr, 'avg'), pw, dw, db, pad)
        feats.append(np.concatenate([bilinear_ac(mx, H, W), bilinear_ac(av, H, W)], axis=1))
    attn = sigmoid(conv1x1(np.concatenate(feats, axis=1), inputs['fusion_w'], inputs['fusion_b']))
    spa_mask = x * attn + conv1x1(x, inputs['resid_w'], inputs['resid_b'])
    # --- CALayer ---
    y = x.mean(axis=(2, 3), keepdims=True, dtype=np.float32)
    y = sigmoid(conv1x1(np.maximum(conv1x1(y, inputs['ca_w1'], inputs['ca_b1']), 0.0),
                        inputs['ca_w2'], inputs['ca_b2']))
    spe_mask = x * y
    mask = conv1x1(spa_mask + spe_mask, inputs['conv1x1_w'], inputs['conv1x1_b']) + x

    # --- LSH bucketing + stable sort (host; permutation only) ---
    xe = conv1x1(mask, inputs['match_w'], inputs['match_b']).reshape(B, CR, L).transpose(0, 2, 1)
    ye = conv1x1(mask, inputs['asm_w'], inputs['asm_b']).reshape(B, C, L).transpose(0, 2, 1)
    rv = np.einsum('blf,fhi->bhli', xe, inputs['rot'].astype(np.float32), dtype=np.float32)
    rv = np.concatenate([rv, -rv], axis=-1)
    codes = rv.argmax(-1).astype(np.int32)          # [B, 4, L]

    in_maps = []
    idxs = []
    for n in range(B):
        for h in range(N_HASHES):
            idx = np.argsort(codes[n, h], kind='stable').astype(np.int64)
            idxs.append(idx)
            xs = xe[n, idx]                          # [L,16] sorted queries
            norm = np.maximum(np.sqrt((xs * xs).sum(-1, dtype=np.float32)), EPS)
            xn = xs / norm[:, None]
            ys = ye[n, idx]                          # [L,64]
            kt = np.concatenate([xn[-CHUNK:], xn, xn[:CHUNK]], axis=0)   # [KW,16]
            v3 = np.concatenate([ys[-CHUNK:], ys, ys[:CHUNK]], axis=0)   # [KW,64]
            v3 = np.concatenate([v3, np.ones((KW, 1), np.float32)], axis=1)
            in_maps.append({
                "qt": np.ascontiguousarray(xs.T).astype(BF16),
                "kt": np.ascontiguousarray(kt.T).astype(BF16),
                "v3": v3.astype(BF16),
            })

    from concourse.bass_utils import run_bass_kernel_spmd
    nc = get_compiled()
    res = run_bass_kernel_spmd(nc, in_maps, list(range(NCORES)), trace=trace)

    # --- unsort + combine across hash rounds (host) ---
    out = np.empty_like(x)
    exec_ns = getattr(res, 'exec_time_ns', None)
    for n in range(B):
        evs = np.zeros((L, C), np.float32)
        ssum = np.zeros((L,), np.float32)
        for h in range(N_HASHES):
            core = n * N_HASHES + h
            evt = np.asarray(res.results[core]["evt"], np.float32)    # [65, L] sorted
            idx = idxs[core]
            evs[idx] += evt[:C].T
            ssum[idx] += evt[C]
        attn_o = evs / ssum[:, None]
        fea = attn_o.T.reshape(1, C, H, W) * RES_SCALE + mask[n:n + 1]
        out[n] = (conv1x1(fea, inputs['collect_w'], inputs['collect_b']) + x[n:n + 1])[0]
    kernel.last_exec_ns = exec_ns
    return out


kernel.last_exec_ns = None



# revision 28
# speedup vs baseline: 1.7750x; 1.7750x over previous
"""Trainium2 kernel for nn_AttentionSparseMask.

Strategy: 8 NeuronCores, data-parallel over (batch n in {0,1}) x (hash round h in
{0..3}).  The dominant compute -- the LSH-chunked sparse attention (S = Q@K^T,
exp, P@V and row sums) -- runs on the NeuronCores via a Bass/Tile kernel.  The
host prepares the sorted/gathered operands (cheap, bandwidth-only) and applies
the small surrounding convolutions.

Device kernel structure (per core, 32 chunks of 512 queries):
  mm1:  S[keys,q] = kt^T-blocks @ qt  (bf16, 12 matmuls/chunk of [128,512])
  exp:  P = exp(S) into fp8e5m2, split ACT (exact exp) / DVE (Schraudolph
        bit-trick producing e5m2 bit patterns via int8 tensor_scalar)
  mm2:  out[q,65] += P^T @ V3 using fp8 DoubleRow matmuls (K=256/instr),
        all 4 query-subtiles accumulate in ONE psum bank (has_written bits)
  copy: [128,260] f32->bf16 on DVE, then DMA to DRAM
"""

import numpy as np
import ml_dtypes

BF16 = ml_dtypes.bfloat16
E5M2 = ml_dtypes.float8_e5m2
E4M3 = ml_dtypes.float8_e4m3

C = 64
RED = 4
CR = C // RED          # 16
N_HASHES = 4
CHUNK = 512
RES_SCALE = 0.1
EPS = 5e-5
H = W = 128
L = H * W              # 16384
NCH = L // CHUNK       # 32 chunks
KW = L + 2 * CHUNK     # wrapped key length 17408
NT = KW // 128         # 136 key tiles
NCORES = 8

# Schraudolph constants for e5m2 bit-pattern exp on DVE (HW rounds RNE):
#   bits_i8 = rint(s * 4/ln2 + (60 - 0.25))
EXP_MUL = 4 / np.log(2.0)        # 5.770780
EXP_ADD = 59.75

_compiled = None


# ----------------------------------------------------------------- host convs
def conv1x1(x, w, b=None):
    out = np.einsum('oc,bchw->bohw', w[:, :, 0, 0], x, dtype=np.float32)
    if b is not None:
        out = out + b[None, :, None, None]
    return out.astype(np.float32)


def dwconv(x, w, b, pad):
    Bb, Cc, Hh, Ww = x.shape
    k = w.shape[2]
    xp = np.pad(x, ((0, 0), (0, 0), (pad, pad), (pad, pad)))
    out = np.zeros((Bb, Cc, Hh + 2 * pad - k + 1, Ww + 2 * pad - k + 1), np.float32)
    for dy in range(k):
        for dx in range(k):
            out += w[None, :, 0, dy, dx, None, None] * \
                xp[:, :, dy:dy + out.shape[2], dx:dx + out.shape[3]]
    if b is not None:
        out = out + b[None, :, None, None]
    return out


def ds_conv(x, pw_w, dw_w, dw_b, pad):
    return dwconv(conv1x1(x, pw_w), dw_w, dw_b, pad)


def pool2(x, mode):
    Bb, Cc, Hh, Ww = x.shape
    xr = x.reshape(Bb, Cc, Hh // 2, 2, Ww // 2, 2)
    return xr.max(axis=(3, 5)) if mode == 'max' else xr.mean(axis=(3, 5), dtype=np.float32)


def bilinear_ac(x, out_h, out_w):
    Bb, Cc, h, w = x.shape
    def coords(n_in, n_out):
        pos = (np.arange(n_out, dtype=np.float32) * np.float32((n_in - 1) / (n_out - 1)))
        lo = np.floor(pos).astype(np.int32)
        hi = np.minimum(lo + 1, n_in - 1)
        frac = (pos - lo.astype(np.float32)).astype(np.float32)
        return lo, hi, frac
    lo_h, hi_h, fh = coords(h, out_h)
    x = x[:, :, lo_h, :] * (1 - fh)[None, None, :, None] + x[:, :, hi_h, :] * fh[None, None, :, None]
    lo_w, hi_w, fw = coords(w, out_w)
    x = x[:, :, :, lo_w] * (1 - fw) + x[:, :, :, hi_w] * fw
    return x.astype(np.float32)


def sigmoid(x):
    return (1.0 / (1.0 + np.exp(-x.astype(np.float32)))).astype(np.float32)


# ------------------------------------------------------------- device kernel
def build_bass():
    import concourse.bass as bass
    import concourse.mybir as mybir
    import concourse.tile as tile
    from concourse import bacc

    nc = bacc.Bacc("TRN2", target_bir_lowering=False)
    f32 = mybir.dt.float32
    bf16 = mybir.dt.bfloat16
    fp8 = mybir.dt.float8e5
    i8 = mybir.dt.int8

    fp8q = mybir.dt.float8e4
    # DoubleRow operand layouts: contraction 16 split as [8 partitions, 2 subtiles]
    qt_d = nc.dram_tensor("qt", [8, NCH, 2, CHUNK], fp8q, kind="ExternalInput")
    kt_d = nc.dram_tensor("kt", [8, NT, 2, 128], fp8q, kind="ExternalInput")
    v3_d = nc.dram_tensor("v3", [128, NT, C + 1], fp8, kind="ExternalInput")
    evt_d = nc.dram_tensor("evt", [NCH, 128, 4, C + 1], bf16, kind="ExternalOutput")

    with tile.TileContext(nc) as tc:
        with (
            tc.tile_pool(name="const", bufs=1) as cpool,
            tc.tile_pool(name="ps", bufs=3, space="PSUM") as pspool,
            tc.tile_pool(name="pr", bufs=2, space="PSUM") as prpool,
            tc.tile_pool(name="pt", bufs=3) as ptpool,
            tc.tile_pool(name="ev", bufs=16) as evpool,
        ):
            qt = cpool.tile([8, NCH, 2, CHUNK], fp8q, tag="qt")
            kt = cpool.tile([8, NT, 2, 128], fp8q, tag="kt")
            v3 = cpool.tile([128, NT, C + 1], fp8, tag="v3")
            # Split input loads into column pieces so chunk 0's operands land
            # fast (V1 DMA cost is per-partition free bytes; pieces pipeline
            # with compute instead of serializing up front).
            # chunk-0 operands first (tiny pieces), then the rest
            nc.sync.dma_start(out=qt[:, 0:1], in_=qt_d[:, 0:1])
            nc.sync.dma_start(out=kt[:, 0:12], in_=kt_d[:, 0:12])
            nc.sync.dma_start(out=qt[:, 1:2], in_=qt_d[:, 1:2])
            nc.sync.dma_start(out=kt[:, 12:16], in_=kt_d[:, 12:16])
            for p in range(1, 16):
                nc.sync.dma_start(out=qt[:, 2 * p:2 * p + 2],
                                  in_=qt_d[:, 2 * p:2 * p + 2])
                kp0 = max(16, (NT * p) // 16)
                kp1 = (NT * (p + 1)) // 16
                if kp1 > kp0:
                    nc.sync.dma_start(out=kt[:, kp0:kp1], in_=kt_d[:, kp0:kp1])
                if p % 4 == 3:
                    v = p // 4
                    nc.sync.dma_start(out=v3[:, v * 34:(v + 1) * 34, :],
                                      in_=v3_d[:, v * 34:(v + 1) * 34, :])

            def emit_mm2_part(c, pt, pr, kts):
                # plain fp8 matmuls: all 4 j-groups share one PSUM bank
                # (has_written bits; start pending-zeroes the whole bank)
                for kt_i in kts:
                    for j in range(4):
                        nc.tensor.matmul(
                            out=pr[:, j * 65:(j + 1) * 65],
                            lhsT=pt[:, kt_i * 512 + j * 128:
                                    kt_i * 512 + (j + 1) * 128],
                            rhs=v3[:, c * 4 + kt_i, :],
                            start=(kt_i == 0 and j == 0),
                            stop=(kt_i == 11 and j == 3),
                            skip_group_check=True,
                        )

            def emit_ev(c, pr):
                ev = evpool.tile([128, 4 * 65], bf16, tag="ev")
                nc.vector.tensor_copy(ev[:], pr[:, :4 * 65])
                nc.sync.dma_start(out=evt_d[c, :, :, :], in_=ev[:])

            prev = None
            for c in range(NCH):
                pt = ptpool.tile([128, 12 * 512], fp8, tag="pt")
                for t in range(6):
                    ps = pspool.tile([128, 1024], f32, tag="ps")
                    for i in range(2):
                        kb = 2 * t + i
                        nc.tensor.matmul(
                            out=ps[:, i * 512:(i + 1) * 512],
                            lhsT=kt[:, c * 4 + kb],      # [8, 2, 128] contiguous
                            rhs=qt[:, c],                # [8, 2, 512] contiguous
                            start=True, stop=True,
                            perf_mode=mybir.MatmulPerfMode.DoubleRow,
                        )
                    g = c * 6 + t
                    dst = pt[:, t * 1024:(t + 1) * 1024]
                    if g % 7 in (0, 2, 4, 6):
                        # ACT: exact exp, RNE-quantized to e5m2 on write
                        nc.scalar.activation(dst, ps[:], mybir.ActivationFunctionType.Exp)
                    else:
                        # DVE: Schraudolph e5m2 bit pattern via int8 affine
                        nc.vector.tensor_scalar(
                            out=dst.bitcast(i8), in0=ps[:],
                            scalar1=EXP_MUL, scalar2=EXP_ADD,
                            op0=mybir.AluOpType.mult, op1=mybir.AluOpType.add,
                        )
                    # interleave previous chunk's mm2: ktiles 0-5 only need
                    # exps t0-t2 of the previous chunk, 6-11 need t3-t5;
                    # the copy lands early in the engine queues
                    if prev is not None:
                        if t == 0:
                            prev_pr = prpool.tile([128, 512], f32, tag="pr",
                                                  name=f"pr_{c}")
                            emit_mm2_part(c - 1, prev, prev_pr, range(0, 6))
                        elif t == 3:
                            emit_mm2_part(c - 1, prev, prev_pr, range(6, 12))
                        elif t == 4:
                            emit_ev(c - 1, prev_pr)
                prev = pt
            last_pr = prpool.tile([128, 512], f32, tag="pr", name="pr_last")
            emit_mm2_part(NCH - 1, prev, last_pr, range(0, 6))
            emit_mm2_part(NCH - 1, prev, last_pr, range(6, 12))
            emit_ev(NCH - 1, last_pr)
    nc.finalize()
    return nc


def get_compiled():
    global _compiled
    if _compiled is None:
        _compiled = build_bass()
    return _compiled


# ------------------------------------------------------------------- kernel
def kernel(trace=False, **inputs):
    x = np.asarray(inputs['x'], np.float32)
    B = x.shape[0]

    # --- MultiScaleSpatialAttention (host, ~50 MFLOP) ---
    xr = conv1x1(x, inputs['spa_down_w'], inputs['spa_down_b'])
    s0 = conv1x1(xr, inputs['s0_pw_w'])
    s0 = s0 * inputs['s0_dw_w'][None, :, 0, 0, 0, None, None] + inputs['s0_dw_b'][None, :, None, None]
    feats = [s0]
    for pw, dw, db, pad in ((inputs['br3_pw_w'], inputs['br3_dw_w'], inputs['br3_dw_b'], 1),
                            (inputs['br5_pw_w'], inputs['br5_dw_w'], inputs['br5_dw_b'], 2),
                            (inputs['br7_pw_w'], inputs['br7_dw_w'], inputs['br7_dw_b'], 3)):
        mx = ds_conv(pool2(xr, 'max'), pw, dw, db, pad)
        av = ds_conv(pool2(xr, 'avg'), pw, dw, db, pad)
        feats.append(np.concatenate([bilinear_ac(mx, H, W), bilinear_ac(av, H, W)], axis=1))
    attn = sigmoid(conv1x1(np.concatenate(feats, axis=1), inputs['fusion_w'], inputs['fusion_b']))
    spa_mask = x * attn + conv1x1(x, inputs['resid_w'], inputs['resid_b'])
    # --- CALayer ---
    y = x.mean(axis=(2, 3), keepdims=True, dtype=np.float32)
    y = sigmoid(conv1x1(np.maximum(conv1x1(y, inputs['ca_w1'], inputs['ca_b1']), 0.0),
                        inputs['ca_w2'], inputs['ca_b2']))
    spe_mask = x * y
    mask = conv1x1(spa_mask + spe_mask, inputs['conv1x1_w'], inputs['conv1x1_b']) + x

    # --- LSH bucketing + stable sort (host; permutation only) ---
    xe = conv1x1(mask, inputs['match_w'], inputs['match_b']).reshape(B, CR, L).transpose(0, 2, 1)
    ye = conv1x1(mask, inputs['asm_w'], inputs['asm_b']).reshape(B, C, L).transpose(0, 2, 1)
    rv = np.einsum('blf,fhi->bhli', xe, inputs['rot'].astype(np.float32), dtype=np.float32)
    rv = np.concatenate([rv, -rv], axis=-1)
    codes = rv.argmax(-1).astype(np.int32)          # [B, 4, L]

    in_maps = []
    idxs = []
    for n in range(B):
        for h in range(N_HASHES):
            idx = np.argsort(codes[n, h], kind='stable').astype(np.int64)
            idxs.append(idx)
            xs = xe[n, idx]                          # [L,16] sorted queries
            norm = np.maximum(np.sqrt((xs * xs).sum(-1, dtype=np.float32)), EPS)
            xn = xs / norm[:, None]
            ys = ye[n, idx]                          # [L,64]
            ktm = np.concatenate([xn[-CHUNK:], xn, xn[:CHUNK]], axis=0)  # [KW,16]
            v3m = np.concatenate([ys[-CHUNK:], ys, ys[:CHUNK]], axis=0)  # [KW,64]
            v3m = np.concatenate([v3m, np.ones((KW, 1), np.float32)], axis=1)
            # SBUF layout [128, NT, 65]: partition = index within 128-key tile
            v3m = np.ascontiguousarray(
                v3m.reshape(NT, 128, C + 1).transpose(1, 0, 2)).astype(E5M2)
            # DoubleRow layouts: [8, NCH, 2, 512] / [8, NT, 2, 128]
            # qt8[p, c, s, q] = xs[c*512+q, 8s+p]
            qt8 = np.ascontiguousarray(
                xs.reshape(NCH, CHUNK, 2, 8).transpose(3, 0, 2, 1)).astype(E4M3)
            kt8 = np.ascontiguousarray(
                ktm.reshape(NT, 128, 2, 8).transpose(3, 0, 2, 1)).astype(E4M3)
            in_maps.append({
                "qt": qt8,
                "kt": kt8,
                "v3": v3m,
            })

    from concourse.bass_utils import run_bass_kernel_spmd
    nc = get_compiled()
    res = run_bass_kernel_spmd(nc, in_maps, list(range(NCORES)), trace=trace)

    # --- unsort + combine across hash rounds (host) ---
    out = np.empty_like(x)
    exec_ns = getattr(res, 'exec_time_ns', None)
    for n in range(B):
        evs = np.zeros((L, C), np.float32)
        ssum = np.zeros((L,), np.float32)
        for h in range(N_HASHES):
            core = n * N_HASHES + h
            evt = np.asarray(res.results[core]["evt"]).astype(np.float32)
            # [NCH, 128, 4, 65] -> sorted row r = c*512 + j*128 + q
            evt = evt.transpose(0, 2, 1, 3).reshape(L, C + 1)
            idx = idxs[core]
            evs[idx] += evt[:, :C]
            ssum[idx] += evt[:, C]
        attn_o = evs / ssum[:, None]
        fea = attn_o.T.reshape(1, C, H, W) * RES_SCALE + mask[n:n + 1]
        out[n] = (conv1x1(fea, inputs['collect_w'], inputs['collect_b']) + x[n:n + 1])[0]
    kernel.last_exec_ns = exec_ns
    return out


kernel.last_exec_ns = None


# revision 35
# speedup vs baseline: 1.7952x; 1.0114x over previous
"""Trainium2 kernel for nn_AttentionSparseMask.

Strategy: 8 NeuronCores, data-parallel over (batch n in {0,1}) x (hash round h in
{0..3}).  The dominant compute -- the LSH-chunked sparse attention (S = Q@K^T,
exp, P@V and row sums) -- runs on the NeuronCores via a Bass/Tile kernel.  The
host prepares the sorted/gathered operands (cheap, bandwidth-only) and applies
the small surrounding convolutions.

Device kernel structure (per core, 32 chunks of 512 queries):
  mm1:  S[keys,q] = kt^T-blocks @ qt  (bf16, 12 matmuls/chunk of [128,512])
  exp:  P = exp(S) into fp8e5m2, split ACT (exact exp) / DVE (Schraudolph
        bit-trick producing e5m2 bit patterns via int8 tensor_scalar)
  mm2:  out[q,65] += P^T @ V3 using fp8 DoubleRow matmuls (K=256/instr),
        all 4 query-subtiles accumulate in ONE psum bank (has_written bits)
  copy: [128,260] f32->bf16 on DVE, then DMA to DRAM
"""

import numpy as np
import ml_dtypes

BF16 = ml_dtypes.bfloat16
E5M2 = ml_dtypes.float8_e5m2
E4M3 = ml_dtypes.float8_e4m3

C = 64
RED = 4
CR = C // RED          # 16
N_HASHES = 4
CHUNK = 512
RES_SCALE = 0.1
EPS = 5e-5
H = W = 128
L = H * W              # 16384
NCH = L // CHUNK       # 32 chunks
KW = L + 2 * CHUNK     # wrapped key length 17408
NT = KW // 128         # 136 key tiles
NCORES = 8

# Schraudolph constants for e5m2 bit-pattern exp on DVE (HW rounds RNE):
#   bits_i8 = rint(s * 4/ln2 + (60 - 0.25))
EXP_MUL = 4 / np.log(2.0)        # 5.770780
EXP_ADD = 59.75

_compiled = None


# ----------------------------------------------------------------- host convs
def conv1x1(x, w, b=None):
    out = np.einsum('oc,bchw->bohw', w[:, :, 0, 0], x, dtype=np.float32)
    if b is not None:
        out = out + b[None, :, None, None]
    return out.astype(np.float32)


def dwconv(x, w, b, pad):
    Bb, Cc, Hh, Ww = x.shape
    k = w.shape[2]
    xp = np.pad(x, ((0, 0), (0, 0), (pad, pad), (pad, pad)))
    out = np.zeros((Bb, Cc, Hh + 2 * pad - k + 1, Ww + 2 * pad - k + 1), np.float32)
    for dy in range(k):
        for dx in range(k):
            out += w[None, :, 0, dy, dx, None, None] * \
                xp[:, :, dy:dy + out.shape[2], dx:dx + out.shape[3]]
    if b is not None:
        out = out + b[None, :, None, None]
    return out


def ds_conv(x, pw_w, dw_w, dw_b, pad):
    return dwconv(conv1x1(x, pw_w), dw_w, dw_b, pad)


def pool2(x, mode):
    Bb, Cc, Hh, Ww = x.shape
    xr = x.reshape(Bb, Cc, Hh // 2, 2, Ww // 2, 2)
    return xr.max(axis=(3, 5)) if mode == 'max' else xr.mean(axis=(3, 5), dtype=np.float32)


def bilinear_ac(x, out_h, out_w):
    Bb, Cc, h, w = x.shape
    def coords(n_in, n_out):
        pos = (np.arange(n_out, dtype=np.float32) * np.float32((n_in - 1) / (n_out - 1)))
        lo = np.floor(pos).astype(np.int32)
        hi = np.minimum(lo + 1, n_in - 1)
        frac = (pos - lo.astype(np.float32)).astype(np.float32)
        return lo, hi, frac
    lo_h, hi_h, fh = coords(h, out_h)
    x = x[:, :, lo_h, :] * (1 - fh)[None, None, :, None] + x[:, :, hi_h, :] * fh[None, None, :, None]
    lo_w, hi_w, fw = coords(w, out_w)
    x = x[:, :, :, lo_w] * (1 - fw) + x[:, :, :, hi_w] * fw
    return x.astype(np.float32)


def sigmoid(x):
    return (1.0 / (1.0 + np.exp(-x.astype(np.float32)))).astype(np.float32)


# ------------------------------------------------------------- device kernel
def build_bass():
    import concourse.bass as bass
    import concourse.mybir as mybir
    import concourse.tile as tile
    from concourse import bacc

    nc = bacc.Bacc("TRN2", target_bir_lowering=False)
    f32 = mybir.dt.float32
    bf16 = mybir.dt.bfloat16
    fp8 = mybir.dt.float8e5
    i8 = mybir.dt.int8

    fp8q = mybir.dt.float8e4
    # DoubleRow operand layouts: contraction 16 split as [8 partitions, 2 subtiles]
    qt_d = nc.dram_tensor("qt", [8, NCH, 2, CHUNK], fp8q, kind="ExternalInput")
    kt_d = nc.dram_tensor("kt", [8, NT, 2, 128], fp8q, kind="ExternalInput")
    v3_d = nc.dram_tensor("v3", [128, NT, C + 1], fp8, kind="ExternalInput")
    evt_d = nc.dram_tensor("evt", [NCH, 128, 4, C + 1], bf16, kind="ExternalOutput")

    with tile.TileContext(nc) as tc:
        with (
            tc.tile_pool(name="const", bufs=1) as cpool,
            tc.tile_pool(name="ps", bufs=3, space="PSUM") as pspool,
            tc.tile_pool(name="pr", bufs=2, space="PSUM") as prpool,
            tc.tile_pool(name="pt", bufs=3) as ptpool,
            tc.tile_pool(name="ev", bufs=16) as evpool,
        ):
            qt = cpool.tile([8, NCH, 2, CHUNK], fp8q, tag="qt")
            kt = cpool.tile([8, NT, 2, 128], fp8q, tag="kt")
            v3 = cpool.tile([128, NT, C + 1], fp8, tag="v3")
            # Split input loads into column pieces so chunk 0's operands land
            # fast (V1 DMA cost is per-partition free bytes; pieces pipeline
            # with compute instead of serializing up front).
            # chunk-0 operands first (tiny pieces), issued from two engines
            nc.sync.dma_start(out=qt[:, 0:1], in_=qt_d[:, 0:1])
            nc.gpsimd.dma_start(out=kt[:, 0:12], in_=kt_d[:, 0:12])
            nc.sync.dma_start(out=qt[:, 1:4], in_=qt_d[:, 1:4])
            nc.sync.dma_start(out=kt[:, 12:17], in_=kt_d[:, 12:17])
            for p in range(1, 8):
                nc.sync.dma_start(out=qt[:, 4 * p:4 * p + 4],
                                  in_=qt_d[:, 4 * p:4 * p + 4])
                kp0 = max(16, (NT * p) // 8)
                kp1 = (NT * (p + 1)) // 8
                if kp1 > kp0:
                    nc.sync.dma_start(out=kt[:, kp0:kp1], in_=kt_d[:, kp0:kp1])
                if p % 2 == 1:
                    v = p // 2
                    nc.sync.dma_start(out=v3[:, v * 34:(v + 1) * 34, :],
                                      in_=v3_d[:, v * 34:(v + 1) * 34, :])

            def emit_mm2_part(c, pt, pr, kts):
                # plain fp8 matmuls: all 4 j-groups share one PSUM bank
                # (has_written bits; start pending-zeroes the whole bank)
                for kt_i in kts:
                    for j in range(4):
                        nc.tensor.matmul(
                            out=pr[:, j * 65:(j + 1) * 65],
                            lhsT=pt[:, kt_i * 512 + j * 128:
                                    kt_i * 512 + (j + 1) * 128],
                            rhs=v3[:, c * 4 + kt_i, :],
                            start=(kt_i == 0 and j == 0),
                            stop=(kt_i == 11 and j == 3),
                            skip_group_check=True,
                        )

            def emit_ev(c, pr):
                ev = evpool.tile([128, 4 * 65], bf16, tag="ev")
                nc.vector.tensor_copy(ev[:], pr[:, :4 * 65])
                nc.sync.dma_start(out=evt_d[c, :, :, :], in_=ev[:])

            prev = None
            for c in range(NCH):
                pt = ptpool.tile([128, 12 * 512], fp8, tag="pt")
                for t in range(6):
                    ps = pspool.tile([128, 1024], f32, tag="ps")
                    for i in range(2):
                        kb = 2 * t + i
                        nc.tensor.matmul(
                            out=ps[:, i * 512:(i + 1) * 512],
                            lhsT=kt[:, c * 4 + kb],      # [8, 2, 128] contiguous
                            rhs=qt[:, c],                # [8, 2, 512] contiguous
                            start=True, stop=True,
                            perf_mode=mybir.MatmulPerfMode.DoubleRow,
                        )
                    g = c * 6 + t
                    dst = pt[:, t * 1024:(t + 1) * 1024]
                    if g % 7 in (0, 2, 4, 6):
                        # ACT: exact exp, RNE-quantized to e5m2 on write
                        nc.scalar.activation(dst, ps[:], mybir.ActivationFunctionType.Exp)
                    else:
                        # DVE: Schraudolph e5m2 bit pattern via int8 affine
                        nc.vector.tensor_scalar(
                            out=dst.bitcast(i8), in0=ps[:],
                            scalar1=EXP_MUL, scalar2=EXP_ADD,
                            op0=mybir.AluOpType.mult, op1=mybir.AluOpType.add,
                        )
                    # interleave previous chunk's mm2: ktiles 0-5 only need
                    # exps t0-t2 of the previous chunk, 6-11 need t3-t5;
                    # the copy lands early in the engine queues
                    if prev is not None:
                        if t == 0:
                            prev_pr = prpool.tile([128, 512], f32, tag="pr",
                                                  name=f"pr_{c}")
                            emit_mm2_part(c - 1, prev, prev_pr, range(0, 6))
                        elif t == 3:
                            emit_mm2_part(c - 1, prev, prev_pr, range(6, 12))
                        elif t == 4:
                            emit_ev(c - 1, prev_pr)
                    if c == NCH - 1 and t == 4:
                        last_pr = prpool.tile([128, 512], f32, tag="pr",
                                              name="pr_last")
                        emit_mm2_part(c, pt, last_pr, range(0, 6))
                prev = pt
            emit_mm2_part(NCH - 1, prev, last_pr, range(6, 12))
            emit_ev(NCH - 1, last_pr)
    nc.finalize()
    return nc


def get_compiled():
    global _compiled
    if _compiled is None:
        _compiled = build_bass()
    return _compiled


# ------------------------------------------------------------------- kernel
def kernel(trace=False, **inputs):
    x = np.asarray(inputs['x'], np.float32)
    B = x.shape[0]

    # --- MultiScaleSpatialAttention (host, ~50 MFLOP) ---
    xr = conv1x1(x, inputs['spa_down_w'], inputs['spa_down_b'])
    s0 = conv1x1(xr, inputs['s0_pw_w'])
    s0 = s0 * inputs['s0_dw_w'][None, :, 0, 0, 0, None, None] + inputs['s0_dw_b'][None, :, None, None]
    feats = [s0]
    for pw, dw, db, pad in ((inputs['br3_pw_w'], inputs['br3_dw_w'], inputs['br3_dw_b'], 1),
                            (inputs['br5_pw_w'], inputs['br5_dw_w'], inputs['br5_dw_b'], 2),
                            (inputs['br7_pw_w'], inputs['br7_dw_w'], inputs['br7_dw_b'], 3)):
        mx = ds_conv(pool2(xr, 'max'), pw, dw, db, pad)
        av = ds_conv(pool2(xr, 'avg'), pw, dw, db, pad)
        feats.append(np.concatenate([bilinear_ac(mx, H, W), bilinear_ac(av, H, W)], axis=1))
    attn = sigmoid(conv1x1(np.concatenate(feats, axis=1), inputs['fusion_w'], inputs['fusion_b']))
    spa_mask = x * attn + conv1x1(x, inputs['resid_w'], inputs['resid_b'])
    # --- CALayer ---
    y = x.mean(axis=(2, 3), keepdims=True, dtype=np.float32)
    y = sigmoid(conv1x1(np.maximum(conv1x1(y, inputs['ca_w1'], inputs['ca_b1']), 0.0),
                        inputs['ca_w2'], inputs['ca_b2']))
    spe_mask = x * y
    mask = conv1x1(spa_mask + spe_mask, inputs['conv1x1_w'], inputs['conv1x1_b']) + x

    # --- LSH bucketing + stable sort (host; permutation only) ---
    xe = conv1x1(mask, inputs['match_w'], inputs['match_b']).reshape(B, CR, L).transpose(0, 2, 1)
    ye = conv1x1(mask, inputs['asm_w'], inputs['asm_b']).reshape(B, C, L).transpose(0, 2, 1)
    rv = np.einsum('blf,fhi->bhli', xe, inputs['rot'].astype(np.float32), dtype=np.float32)
    rv = np.concatenate([rv, -rv], axis=-1)
    codes = rv.argmax(-1).astype(np.int32)          # [B, 4, L]

    in_maps = []
    idxs = []
    for n in range(B):
        for h in range(N_HASHES):
            idx = np.argsort(codes[n, h], kind='stable').astype(np.int64)
            idxs.append(idx)
            xs = xe[n, idx]                          # [L,16] sorted queries
            norm = np.maximum(np.sqrt((xs * xs).sum(-1, dtype=np.float32)), EPS)
            xn = xs / norm[:, None]
            ys = ye[n, idx]                          # [L,64]
            ktm = np.concatenate([xn[-CHUNK:], xn, xn[:CHUNK]], axis=0)  # [KW,16]
            v3m = np.concatenate([ys[-CHUNK:], ys, ys[:CHUNK]], axis=0)  # [KW,64]
            v3m = np.concatenate([v3m, np.ones((KW, 1), np.float32)], axis=1)
            # SBUF layout [128, NT, 65]: partition = index within 128-key tile
            v3m = np.ascontiguousarray(
                v3m.reshape(NT, 128, C + 1).transpose(1, 0, 2)).astype(E5M2)
            # DoubleRow layouts: [8, NCH, 2, 512] / [8, NT, 2, 128]
            # qt8[p, c, s, q] = xs[c*512+q, 8s+p]
            qt8 = np.ascontiguousarray(
                xs.reshape(NCH, CHUNK, 2, 8).transpose(3, 0, 2, 1)).astype(E4M3)
            kt8 = np.ascontiguousarray(
                ktm.reshape(NT, 128, 2, 8).transpose(3, 0, 2, 1)).astype(E4M3)
            in_maps.append({
                "qt": qt8,
                "kt": kt8,
                "v3": v3m,
            })

    from concourse.bass_utils import run_bass_kernel_spmd
    nc = get_compiled()
    res = run_bass_kernel_spmd(nc, in_maps, list(range(NCORES)), trace=trace)

    # --- unsort + combine across hash rounds (host) ---
    out = np.empty_like(x)
    exec_ns = getattr(res, 'exec_time_ns', None)
    for n in range(B):
        evs = np.zeros((L, C), np.float32)
        ssum = np.zeros((L,), np.float32)
        for h in range(N_HASHES):
            core = n * N_HASHES + h
            evt = np.asarray(res.results[core]["evt"]).astype(np.float32)
            # [NCH, 128, 4, 65] -> sorted row r = c*512 + j*128 + q
            evt = evt.transpose(0, 2, 1, 3).reshape(L, C + 1)
            idx = idxs[core]
            evs[idx] += evt[:, :C]
            ssum[idx] += evt[:, C]
        attn_o = evs / ssum[:, None]
        fea = attn_o.T.reshape(1, C, H, W) * RES_SCALE + mask[n:n + 1]
        out[n] = (conv1x1(fea, inputs['collect_w'], inputs['collect_b']) + x[n:n + 1])[0]
    kernel.last_exec_ns = exec_ns
    return out


kernel.last_exec_ns = None
